# revision 21
# baseline (speedup 1.0000x reference)
"""Binary Matching Pursuit kernel for Trainium2 (8 NeuronCores).

Data-parallel over batch B=512 -> 64 rows/core; weight [2048, 512]
replicated. Each core runs the 21-step greedy pursuit loop on device.

Decisions match the fp32 reference exactly:
  - residual_t = R0 - xr_t @ W^T (+ masking of already-picked cols), with
    R0 = 2x @ W^T computed once in exact fp32 matmuls.
  - xr is binary, so xr @ W^T is computed as xr @ (Wh + Wl)^T with
    Wh = bf16(W^T), Wl = bf16(W^T - Wh): products are exact (x1.0) and
    the split truncation is ~2^-18 relative -> fp32-class accuracy at
    bf16 matmul speed.
  - The -lambd*encoded term in the reference only excludes picked columns
    from the argmax; we instead zero R0pen at picked positions (masked
    values stay < 0.5 while the true row max is ~4, so decisions are
    unchanged).
  - encoded is reconstructed by scattering ones at the 21*64 picked
    positions.

Layout: the residual pipeline runs in a split layout [128, 1024]:
partition p < 64 holds row b=p, output cols [0, 1024); partition p >= 64
holds row b=p-64, cols [1024, 2048). This uses all 128 vector lanes and
both halves of the PE array (col-tiling at tile_position (0,64)); a
small PE shift-matmul merges the per-half argmax candidates.
"""

import contextlib
import numpy as np

B_CORE = 64
N_IN = 512
N_OUT = 2048
HALF = N_OUT // 2
K_ACTIVE = 21  # ceil(0.01 * 2048)
K_IN = 6       # ceil(0.01 * 512)
N_CORES = 8

_CACHED_NC = None


def _build(reps=1, skip=(), loop_scope="pursuit"):
    import concourse.bacc as bacc
    import concourse.mybir as mybir
    from concourse import bass
    from concourse.tile import TileContext
    from concourse.masks import make_identity

    f32 = mybir.dt.float32
    b16 = mybir.dt.bfloat16
    i32 = mybir.dt.int32
    u32 = mybir.dt.uint32
    u8 = mybir.dt.uint8
    Alu = mybir.AluOpType

    nc = bacc.Bacc("TRN2", target_bir_lowering=False)
    x_d = nc.dram_tensor("x", [B_CORE, N_IN], f32, kind="ExternalInput")
    w_d = nc.dram_tensor("w", [N_OUT, N_IN], f32, kind="ExternalInput")
    enc_d = nc.dram_tensor("enc", [B_CORE * N_OUT, 1], f32, kind="ExternalOutput")
    xr_d = nc.dram_tensor("xr", [B_CORE, N_IN], f32, kind="ExternalOutput")

    with TileContext(nc) as tc:
        with (
            tc.tile_pool(name="const", bufs=1) as cpool,
            tc.tile_pool(name="wpool", bufs=1) as wpool,
            tc.tile_pool(name="state", bufs=1) as spool,
            tc.tile_pool(name="scratch", bufs=2) as scr,
            tc.tile_pool(name="psum", bufs=1, space="PSUM") as pp,
            tc.tile_pool(name="psum_tr", bufs=3, space="PSUM") as ptr,
            tc.tile_pool(name="psum_mg", bufs=1, space="PSUM") as pmg,
        ):
            # ---------------- constants ----------------
            ident_f = cpool.tile([128, 128], f32)
            make_identity(nc, ident_f[:])
            ident_b = cpool.tile([128, 128], b16)
            nc.vector.tensor_copy(ident_b[:], ident_f[:])

            # shiftS[64+b, b] = 1 : moves partitions 64..127 down to 0..63
            shiftS = cpool.tile([128, B_CORE], f32)
            nc.gpsimd.memset(shiftS[:], 0.0)
            nc.gpsimd.affine_select(
                out=shiftS[:], in_=shiftS[:], compare_op=Alu.not_equal,
                fill=1.0, base=-B_CORE, channel_multiplier=1,
                pattern=[[-1, B_CORE]])
            # repS[b, b] = repS[b, 64+b] = 1 : replicates rows to both halves
            repS = cpool.tile([B_CORE, 128], f32)
            nc.gpsimd.memset(repS[:], 0.0)
            nc.gpsimd.affine_select(
                out=repS[:], in_=repS[:], compare_op=Alu.not_equal,
                fill=1.0, base=0, channel_multiplier=1, pattern=[[-1, 128]])
            nc.gpsimd.affine_select(
                out=repS[:], in_=repS[:], compare_op=Alu.not_equal,
                fill=1.0, base=B_CORE, channel_multiplier=1,
                pattern=[[-1, 128]])

            # iota over global output cols in split layout
            iota_i = cpool.tile([128, HALF], i32)
            nc.gpsimd.iota(iota_i[:], pattern=[[1, HALF]], base=0,
                           channel_multiplier=0)
            iota_f = cpool.tile([128, HALF], f32)
            nc.vector.tensor_copy(iota_f[:], iota_i[:])
            nc.vector.tensor_scalar_add(iota_f[B_CORE:, :], iota_f[B_CORE:, :],
                                        float(HALF))

            # off_f[p] = 0 / 1024 per half
            off_f = cpool.tile([128, 1], f32)
            nc.vector.memset(off_f[:B_CORE, :], 0.0)
            nc.vector.memset(off_f[B_CORE:, :], float(HALF))

            rowbase = cpool.tile([B_CORE, 1], i32)
            nc.gpsimd.iota(rowbase[:], pattern=[[0, 1]], base=0,
                           channel_multiplier=N_OUT)
            ones = cpool.tile([B_CORE, 1], f32)
            nc.vector.memset(ones[:], 1.0)

            full_loop_cm = (tc.For_i(0, reps)
                            if reps > 1 and loop_scope == "full"
                            else contextlib.nullcontext())
            full_loop_cm.__enter__()

            # ---------------- load x, build 2*x^T ----------------
            x_sb = scr.tile([B_CORE, N_IN], f32, tag="xload")
            nc.sync.dma_start(out=x_sb[:], in_=x_d[:])
            x2t = spool.tile([128, 4, B_CORE], f32)
            for b in range(4):
                pst = ptr.tile([128, 128], f32, tag="tr")
                nc.tensor.transpose(pst[:, :B_CORE],
                                    x_sb[:, 128 * b:128 * b + 128],
                                    ident_f[:B_CORE, :B_CORE])
                nc.scalar.mul(out=x2t[:, b, :], in_=pst[:, :B_CORE], mul=2.0)

            # ---------------- load W, build W^T (fp32) ----------------
            w_nat = wpool.tile([128, 16, N_IN], f32)
            for ot in range(16):
                nc.sync.dma_start(out=w_nat[:, ot, :],
                                  in_=w_d[128 * ot:128 * ot + 128, :])
            wt = wpool.tile([128, 4, N_OUT], f32)
            for ot in range(16):
                for ib in range(4):
                    pst = ptr.tile([128, 128], f32, tag="tr")
                    nc.tensor.transpose(
                        pst[:], w_nat[:, ot, 128 * ib:128 * ib + 128],
                        ident_f[:])
                    if (ot * 4 + ib) % 2 == 0:
                        nc.scalar.copy(out=wt[:, ib, 128 * ot:128 * ot + 128],
                                       in_=pst[:])
                    else:
                        nc.vector.tensor_copy(
                            wt[:, ib, 128 * ot:128 * ot + 128], pst[:])

            # ---------------- split W^T = Wh + Wl (bf16) ----------------
            wh = wpool.tile([128, 4, N_OUT], b16)
            wl = wpool.tile([128, 4, N_OUT], b16)
            for ib in range(4):
                nc.vector.tensor_copy(wh[:, ib, :], wt[:, ib, :])
            for ib in range(4):
                nc.vector.tensor_tensor(
                    out=wl[:, ib, :], in0=wt[:, ib, :], in1=wh[:, ib, :],
                    op=Alu.subtract)

            # ---------------- R0 = 2x @ W^T in split layout ----------------
            r0pen = spool.tile([128, HALF], f32)
            for c in range(2):
                ps_r0 = pp.tile([128, 512], f32, tag=f"mm{c}")
                for k in range(4):
                    nc.tensor.matmul(
                        out=ps_r0[:B_CORE, :],
                        lhsT=x2t[:, k, :],
                        rhs=wt[:, k, 512 * c:512 * c + 512],
                        start=(k == 0), stop=(k == 3))
                    nc.tensor.matmul(
                        out=ps_r0[B_CORE:, :],
                        lhsT=x2t[:, k, :],
                        rhs=wt[:, k, HALF + 512 * c:HALF + 512 * c + 512],
                        start=(k == 0), stop=(k == 3),
                        tile_position=(0, B_CORE))
                nc.scalar.copy(out=r0pen[:, 512 * c:512 * c + 512],
                               in_=ps_r0[:])

            # ---------------- pursuit state ----------------
            y = spool.tile([B_CORE, N_IN], f32)
            nc.vector.memset(y[:], 0.0)
            xrT = spool.tile([128, 4, B_CORE], b16)
            nc.vector.memset(xrT[:], 0.0)
            fpicks = spool.tile([B_CORE, K_ACTIVE + 1], i32)

            if reps > 1 and loop_scope == "pursuit":
                r0 = spool.tile([128, HALF], f32)
                nc.vector.tensor_copy(r0[:], r0pen[:])
                loop_cm = tc.For_i(0, reps)
            else:
                loop_cm = contextlib.nullcontext()

            # ---------------- pursuit loop ----------------
            with loop_cm:
                if reps > 1 and loop_scope == "pursuit":
                    nc.vector.memset(y[:], 0.0)
                    nc.vector.tensor_copy(r0pen[:], r0[:])
                for t in range(K_ACTIVE):
                    last = t == K_ACTIVE - 1
                    m8c = scr.tile([128, 2, 8], f32, tag="m8")
                    i8c = scr.tile([128, 2, 8], u32, tag="i8")
                    if t == 0 or "mm" in skip:
                        for c in range(2):
                            nc.vector.max(out=m8c[:, c, :],
                                          in_=r0pen[:, 512 * c:512 * c + 512])
                            nc.vector.max_index(
                                out=i8c[:, c, :], in_max=m8c[:, c, :],
                                in_values=r0pen[:, 512 * c:512 * c + 512])
                    else:
                        res = scr.tile([128, HALF], f32, tag="res")
                        for c in range(2):
                            psc = pp.tile([128, 512], f32, tag=f"mm{c}")
                            for k in range(4):
                                for wsp in (wh, wl):
                                    nc.tensor.matmul(
                                        out=psc[:B_CORE, :],
                                        lhsT=xrT[:, k, :],
                                        rhs=wsp[:, k, 512 * c:512 * c + 512],
                                        start=(k == 0 and wsp is wh),
                                        stop=(k == 3 and wsp is wl))
                                    nc.tensor.matmul(
                                        out=psc[B_CORE:, :],
                                        lhsT=xrT[:, k, :],
                                        rhs=wsp[:, k,
                                                HALF + 512 * c:HALF + 512 * c + 512],
                                        start=(k == 0 and wsp is wh),
                                        stop=(k == 3 and wsp is wl),
                                        tile_position=(0, B_CORE))
                            # chunk c ready: overlap its reduction with chunk c+1
                            nc.vector.scalar_tensor_tensor(
                                out=res[:, 512 * c:512 * c + 512],
                                in0=psc[:], scalar=-1.0,
                                in1=r0pen[:, 512 * c:512 * c + 512],
                                op0=Alu.mult, op1=Alu.add)
                            nc.vector.max(out=m8c[:, c, :],
                                          in_=res[:, 512 * c:512 * c + 512])
                            nc.vector.max_index(
                                out=i8c[:, c, :], in_max=m8c[:, c, :],
                                in_values=res[:, 512 * c:512 * c + 512])

                    # merge chunk then half candidates per row
                    if "merge" in skip:
                        jf = scr.tile([B_CORE, 1], f32, tag="jf")
                        nc.vector.memset(jf[:], 7.0)
                        ju = scr.tile([B_CORE, 1], u32, tag="ju")
                        nc.vector.memset(ju[:], 7)
                    else:
                        # chunk winner: value + local col (fp32, exact ints)
                        i1p = scr.tile([128, 1], f32, tag="i1p")
                        nc.vector.tensor_scalar_add(i1p[:], i8c[:, 1, 0:1],
                                                    512.0)
                        ccm = scr.tile([128, 1], u8, tag="ccm")
                        nc.vector.tensor_tensor(out=ccm[:],
                                                in0=m8c[:, 0, 0:1],
                                                in1=m8c[:, 1, 0:1],
                                                op=Alu.is_ge)
                        cand = scr.tile([128, 2], f32, tag="cand")
                        nc.vector.select(out=cand[:, 0:1], mask=ccm[:],
                                         on_true=m8c[:, 0, 0:1],
                                         on_false=m8c[:, 1, 0:1])
                        i0f = scr.tile([128, 1], f32, tag="i0f")
                        nc.vector.tensor_copy(i0f[:], i8c[:, 0, 0:1])
                        iloc = scr.tile([128, 1], f32, tag="iloc")
                        nc.vector.select(out=iloc[:], mask=ccm[:],
                                         on_true=i0f[:], on_false=i1p[:])
                        nc.vector.tensor_tensor(out=cand[:, 1:2], in0=iloc[:],
                                                in1=off_f[:], op=Alu.add)
                        ps_hi = pmg.tile([B_CORE, 2], f32, tag="mg")
                        nc.tensor.matmul(out=ps_hi[:], lhsT=shiftS[:],
                                         rhs=cand[:], start=True, stop=True)
                        hi2 = scr.tile([B_CORE, 2], f32, tag="hi2")
                        nc.scalar.copy(out=hi2[:], in_=ps_hi[:])
                        win = scr.tile([B_CORE, 1], u8, tag="win")
                        nc.vector.tensor_tensor(out=win[:],
                                                in0=cand[:B_CORE, 0:1],
                                                in1=hi2[:, 0:1], op=Alu.is_ge)
                        jf = scr.tile([B_CORE, 1], f32, tag="jf")
                        nc.vector.select(out=jf[:], mask=win[:],
                                         on_true=cand[:B_CORE, 1:2],
                                         on_false=hi2[:, 1:2])
                        ju = scr.tile([B_CORE, 1], u32, tag="ju")
                        nc.vector.tensor_copy(ju[:], jf[:])

                    # gather W[j, :] and accumulate into y (DMA compute add)
                    if "gather" not in skip:
                        if "gstage" in skip:
                            wrow = scr.tile([B_CORE, N_IN], f32, tag="wrow")
                            nc.gpsimd.indirect_dma_start(
                                out=wrow[:], out_offset=None, in_=w_d[:],
                                in_offset=bass.IndirectOffsetOnAxis(
                                    ap=ju[:, 0:1], axis=0))
                            nc.vector.tensor_tensor(out=y[:], in0=y[:],
                                                    in1=wrow[:], op=Alu.add)
                        else:
                            nc.gpsimd.indirect_dma_start(
                                out=y[:], out_offset=None, in_=w_d[:],
                                in_offset=bass.IndirectOffsetOnAxis(
                                    ap=ju[:, 0:1], axis=0),
                                compute_op=Alu.add)

                    # flat pick index for the final encoded scatter
                    nc.vector.tensor_tensor(out=fpicks[:, t:t + 1], in0=ju[:],
                                            in1=rowbase[:], op=Alu.add)

                    # top-6 threshold of y
                    m8y = scr.tile([B_CORE, 8], f32, tag="m8y")
                    nc.vector.max(out=m8y[:], in_=y[:])
                    thr = m8y[:, K_IN - 1:K_IN]

                    if not last:
                        # xr mask (bf16); transpose via DMA xbar for next MM
                        xrb = scr.tile([B_CORE, N_IN], b16, tag="xrb")
                        for b in range(4):
                            nc.vector.tensor_tensor(
                                out=xrb[:, 128 * b:128 * b + 128],
                                in0=y[:, 128 * b:128 * b + 128],
                                in1=thr.to_broadcast([B_CORE, 128]),
                                op=Alu.is_ge)
                            if "transpose" not in skip:
                                pst = ptr.tile([128, 128], b16, tag="tr")
                                nc.tensor.transpose(
                                    pst[:, :B_CORE],
                                    xrb[:, 128 * b:128 * b + 128],
                                    ident_b[:B_CORE, :B_CORE])
                                nc.scalar.copy(out=xrT[:, b, :],
                                               in_=pst[:, :B_CORE])

                        # replicate j to both halves, mask r0pen at the pick
                        if "mask" not in skip:
                            ps_rep = pmg.tile([128, 1], f32, tag="mgr")
                            nc.tensor.matmul(out=ps_rep[:], lhsT=repS[:],
                                             rhs=jf[:], start=True, stop=True)
                            jrep = scr.tile([128, 1], f32, tag="jrep")
                            nc.scalar.copy(out=jrep[:], in_=ps_rep[:])
                            nc.vector.scalar_tensor_tensor(
                                out=r0pen[:], in0=iota_f[:], scalar=jrep[:],
                                in1=r0pen[:], op0=Alu.not_equal, op1=Alu.mult)
                    else:
                        xrf = scr.tile([B_CORE, N_IN], f32, tag="xrf")
                        nc.vector.tensor_tensor(
                            out=xrf[:], in0=y[:],
                            in1=thr.to_broadcast([B_CORE, N_IN]), op=Alu.is_ge)
                        nc.sync.dma_start(out=xr_d[:], in_=xrf[:])

            full_loop_cm.__exit__(None, None, None)

            # ---------------- emit encoded ----------------
            for t in range(K_ACTIVE):
                nc.gpsimd.indirect_dma_start(
                    out=enc_d[:],
                    out_offset=bass.IndirectOffsetOnAxis(
                        ap=fpicks[:, t:t + 1], axis=0),
                    in_=ones[:], in_offset=None)

    nc.compile()
    return nc


def _get_nc(reps=1, skip=(), loop_scope="pursuit"):
    global _CACHED_NC
    if reps != 1 or skip:
        return _build(reps, skip, loop_scope)
    if _CACHED_NC is None:
        _CACHED_NC = _build()
    return _CACHED_NC


def kernel(x: np.ndarray, weight: np.ndarray):
    from concourse.bass_utils import run_bass_kernel_spmd

    x = np.ascontiguousarray(x, dtype=np.float32)
    weight = np.ascontiguousarray(weight, dtype=np.float32)
    nc = _get_nc()
    in_maps = [
        {"x": x[c * B_CORE:(c + 1) * B_CORE], "w": weight}
        for c in range(N_CORES)
    ]
    r = run_bass_kernel_spmd(nc, in_maps, list(range(N_CORES)))
    enc = np.concatenate(
        [r.results[c]["enc"].reshape(B_CORE, N_OUT) for c in range(N_CORES)],
        axis=0)
    xr = np.concatenate([r.results[c]["xr"] for c in range(N_CORES)], axis=0)
    return enc, xr


# revision 24
# speedup vs baseline: 1.0174x; 1.0174x over previous
"""Binary Matching Pursuit kernel for Trainium2 (8 NeuronCores).

Data-parallel over batch B=512 -> 64 rows/core; weight [2048, 512]
replicated. Each core runs the 21-step greedy pursuit loop on device.

Decisions match the fp32 reference exactly:
  - residual_t = R0 - xr_t @ W^T (+ masking of already-picked cols), with
    R0 = 2x @ W^T computed once in exact fp32 matmuls.
  - xr is binary, so xr @ W^T is computed as xr @ (Wh + Wl)^T with
    Wh = bf16(W^T), Wl = bf16(W^T - Wh): products are exact (x1.0) and
    the split truncation is ~2^-18 relative -> fp32-class accuracy at
    bf16 matmul speed.
  - The -lambd*encoded term in the reference only excludes picked columns
    from the argmax; we instead zero R0pen at picked positions (masked
    values stay < 0.5 while the true row max is ~4, so decisions are
    unchanged).
  - encoded is reconstructed by scattering ones at the 21*64 picked
    positions.

Layout: the residual pipeline runs in a split layout [128, 1024]:
partition p < 64 holds row b=p, output cols [0, 1024); partition p >= 64
holds row b=p-64, cols [1024, 2048). This uses all 128 vector lanes and
both halves of the PE array (col-tiling at tile_position (0,64)); a
small PE shift-matmul merges the per-half argmax candidates.
"""

import contextlib
import numpy as np

B_CORE = 64
N_IN = 512
N_OUT = 2048
HALF = N_OUT // 2
K_ACTIVE = 21  # ceil(0.01 * 2048)
K_IN = 6       # ceil(0.01 * 512)
N_CORES = 8

_CACHED_NC = None


def _build(reps=1, skip=(), loop_scope="pursuit"):
    import concourse.bacc as bacc
    import concourse.mybir as mybir
    from concourse import bass
    from concourse.tile import TileContext
    from concourse.masks import make_identity

    f32 = mybir.dt.float32
    b16 = mybir.dt.bfloat16
    i32 = mybir.dt.int32
    u32 = mybir.dt.uint32
    u8 = mybir.dt.uint8
    Alu = mybir.AluOpType

    nc = bacc.Bacc("TRN2", target_bir_lowering=False)
    x_d = nc.dram_tensor("x", [B_CORE, N_IN], f32, kind="ExternalInput")
    w_d = nc.dram_tensor("w", [N_OUT, N_IN], f32, kind="ExternalInput")
    enc_d = nc.dram_tensor("enc", [B_CORE * N_OUT, 1], f32, kind="ExternalOutput")
    xr_d = nc.dram_tensor("xr", [B_CORE, N_IN], f32, kind="ExternalOutput")

    with TileContext(nc) as tc:
        with (
            tc.tile_pool(name="const", bufs=1) as cpool,
            tc.tile_pool(name="wpool", bufs=1) as wpool,
            tc.tile_pool(name="state", bufs=1) as spool,
            tc.tile_pool(name="scratch", bufs=2) as scr,
            tc.tile_pool(name="psum", bufs=1, space="PSUM") as pp,
            tc.tile_pool(name="psum_tr", bufs=3, space="PSUM") as ptr,
        ):
            # ---------------- constants ----------------
            ident_f = cpool.tile([128, 128], f32)
            make_identity(nc, ident_f[:])
            ident_b = cpool.tile([128, 128], b16)
            nc.vector.tensor_copy(ident_b[:], ident_f[:])

            # iota over global output cols in split layout
            iota_i = cpool.tile([128, HALF], i32)
            nc.gpsimd.iota(iota_i[:], pattern=[[1, HALF]], base=0,
                           channel_multiplier=0)
            iota_f = cpool.tile([128, HALF], f32)
            nc.vector.tensor_copy(iota_f[:], iota_i[:])
            nc.vector.tensor_scalar_add(iota_f[B_CORE:, :], iota_f[B_CORE:, :],
                                        float(HALF))

            # off_f[p] = 0 / 1024 per half
            off_f = cpool.tile([128, 1], f32)
            nc.vector.memset(off_f[:B_CORE, :], 0.0)
            nc.vector.memset(off_f[B_CORE:, :], float(HALF))

            rowbase = cpool.tile([B_CORE, 1], i32)
            nc.gpsimd.iota(rowbase[:], pattern=[[0, 1]], base=0,
                           channel_multiplier=N_OUT)
            ones = cpool.tile([B_CORE, 1], f32)
            nc.vector.memset(ones[:], 1.0)

            full_loop_cm = (tc.For_i(0, reps)
                            if reps > 1 and loop_scope == "full"
                            else contextlib.nullcontext())
            full_loop_cm.__enter__()

            # ---------------- load x, build 2*x^T ----------------
            x_sb = scr.tile([B_CORE, N_IN], f32, tag="xload")
            nc.sync.dma_start(out=x_sb[:], in_=x_d[:])
            x2t = spool.tile([128, 4, B_CORE], f32)
            for b in range(4):
                pst = ptr.tile([128, 128], f32, tag="tr")
                nc.tensor.transpose(pst[:, :B_CORE],
                                    x_sb[:, 128 * b:128 * b + 128],
                                    ident_f[:B_CORE, :B_CORE])
                nc.scalar.mul(out=x2t[:, b, :], in_=pst[:, :B_CORE], mul=2.0)

            # ---------------- load W, build W^T (fp32) ----------------
            w_nat = wpool.tile([128, 16, N_IN], f32)
            for ot in range(16):
                nc.sync.dma_start(out=w_nat[:, ot, :],
                                  in_=w_d[128 * ot:128 * ot + 128, :])
            wt = wpool.tile([128, 4, N_OUT], f32)
            for ot in range(16):
                for ib in range(4):
                    pst = ptr.tile([128, 128], f32, tag="tr")
                    nc.tensor.transpose(
                        pst[:], w_nat[:, ot, 128 * ib:128 * ib + 128],
                        ident_f[:])
                    if (ot * 4 + ib) % 2 == 0:
                        nc.scalar.copy(out=wt[:, ib, 128 * ot:128 * ot + 128],
                                       in_=pst[:])
                    else:
                        nc.vector.tensor_copy(
                            wt[:, ib, 128 * ot:128 * ot + 128], pst[:])

            # ---------------- split W^T = Wh + Wl (bf16) ----------------
            wh = wpool.tile([128, 4, N_OUT], b16)
            wl = wpool.tile([128, 4, N_OUT], b16)
            for ib in range(4):
                nc.vector.tensor_copy(wh[:, ib, :], wt[:, ib, :])
            for ib in range(4):
                nc.vector.tensor_tensor(
                    out=wl[:, ib, :], in0=wt[:, ib, :], in1=wh[:, ib, :],
                    op=Alu.subtract)

            # ---------------- R0 = 2x @ W^T in split layout ----------------
            r0pen = spool.tile([128, HALF], f32)
            for c in range(2):
                ps_r0 = pp.tile([128, 512], f32, tag=f"mm{c}")
                for k in range(4):
                    nc.tensor.matmul(
                        out=ps_r0[:B_CORE, :],
                        lhsT=x2t[:, k, :],
                        rhs=wt[:, k, 512 * c:512 * c + 512],
                        start=(k == 0), stop=(k == 3))
                    nc.tensor.matmul(
                        out=ps_r0[B_CORE:, :],
                        lhsT=x2t[:, k, :],
                        rhs=wt[:, k, HALF + 512 * c:HALF + 512 * c + 512],
                        start=(k == 0), stop=(k == 3),
                        tile_position=(0, B_CORE))
                nc.scalar.copy(out=r0pen[:, 512 * c:512 * c + 512],
                               in_=ps_r0[:])

            # ---------------- pursuit state ----------------
            y = spool.tile([B_CORE, N_IN], f32)
            nc.vector.memset(y[:], 0.0)
            xrT = spool.tile([128, 4, B_CORE], b16)
            nc.vector.memset(xrT[:], 0.0)
            fpicks = spool.tile([B_CORE, K_ACTIVE + 1], i32)

            if reps > 1 and loop_scope == "pursuit":
                r0 = spool.tile([128, HALF], f32)
                nc.vector.tensor_copy(r0[:], r0pen[:])
                loop_cm = tc.For_i(0, reps)
            else:
                loop_cm = contextlib.nullcontext()

            # ---------------- pursuit loop ----------------
            with loop_cm:
                if reps > 1 and loop_scope == "pursuit":
                    nc.vector.memset(y[:], 0.0)
                    nc.vector.tensor_copy(r0pen[:], r0[:])
                for t in range(K_ACTIVE):
                    last = t == K_ACTIVE - 1
                    m8c = scr.tile([128, 2, 8], f32, tag="m8")
                    i8c = scr.tile([128, 2, 8], u32, tag="i8")
                    if t == 0 or "mm" in skip:
                        for c in range(2):
                            nc.vector.max(out=m8c[:, c, :],
                                          in_=r0pen[:, 512 * c:512 * c + 512])
                            nc.vector.max_index(
                                out=i8c[:, c, :], in_max=m8c[:, c, :],
                                in_values=r0pen[:, 512 * c:512 * c + 512])
                    else:
                        res = scr.tile([128, HALF], f32, tag="res")
                        for c in range(2):
                            psc = pp.tile([128, 512], f32, tag=f"mm{c}")
                            for k in range(4):
                                for wsp in (wh, wl):
                                    nc.tensor.matmul(
                                        out=psc[:B_CORE, :],
                                        lhsT=xrT[:, k, :],
                                        rhs=wsp[:, k, 512 * c:512 * c + 512],
                                        start=(k == 0 and wsp is wh),
                                        stop=(k == 3 and wsp is wl))
                                    nc.tensor.matmul(
                                        out=psc[B_CORE:, :],
                                        lhsT=xrT[:, k, :],
                                        rhs=wsp[:, k,
                                                HALF + 512 * c:HALF + 512 * c + 512],
                                        start=(k == 0 and wsp is wh),
                                        stop=(k == 3 and wsp is wl),
                                        tile_position=(0, B_CORE))
                            # chunk c ready: overlap its reduction with chunk c+1
                            nc.vector.scalar_tensor_tensor(
                                out=res[:, 512 * c:512 * c + 512],
                                in0=psc[:], scalar=-1.0,
                                in1=r0pen[:, 512 * c:512 * c + 512],
                                op0=Alu.mult, op1=Alu.add)
                            nc.vector.max(out=m8c[:, c, :],
                                          in_=res[:, 512 * c:512 * c + 512])
                            nc.vector.max_index(
                                out=i8c[:, c, :], in_max=m8c[:, c, :],
                                in_values=res[:, 512 * c:512 * c + 512])

                    # merge chunk then half candidates per row
                    if "merge" in skip:
                        jf = scr.tile([B_CORE, 1], f32, tag="jf")
                        nc.vector.memset(jf[:], 7.0)
                        ju = scr.tile([B_CORE, 1], u32, tag="ju")
                        nc.vector.memset(ju[:], 7)
                    else:
                        # chunk winner: value + local col (fp32, exact ints).
                        # Pure-DVE chain: partition-shifted copies (verified
                        # on HW) move the upper half's candidate down, so no
                        # PE/ACT hops are needed.
                        i0f = scr.tile([128, 1], f32, tag="i0f")
                        nc.vector.tensor_copy(i0f[:], i8c[:, 0, 0:1])
                        i1p = scr.tile([128, 1], f32, tag="i1p")
                        nc.vector.tensor_scalar_add(i1p[:], i8c[:, 1, 0:1],
                                                    512.0)
                        ccm = scr.tile([128, 1], u8, tag="ccm")
                        nc.vector.tensor_tensor(out=ccm[:],
                                                in0=m8c[:, 0, 0:1],
                                                in1=m8c[:, 1, 0:1],
                                                op=Alu.is_ge)
                        nc.vector.copy_predicated(out=i1p[:], mask=ccm[:],
                                                  data=i0f[:])
                        pair = scr.tile([128, 2], f32, tag="pair")
                        nc.vector.tensor_tensor(out=pair[:, 0:1],
                                                in0=m8c[:, 0, 0:1],
                                                in1=m8c[:, 1, 0:1],
                                                op=Alu.max)
                        nc.vector.tensor_tensor(out=pair[:, 1:2], in0=i1p[:],
                                                in1=off_f[:], op=Alu.add)
                        hi2 = scr.tile([B_CORE, 2], f32, tag="hi2")
                        nc.vector.tensor_copy(hi2[:], pair[B_CORE:, :])
                        win = scr.tile([B_CORE, 1], u8, tag="win")
                        nc.vector.tensor_tensor(out=win[:],
                                                in0=pair[:B_CORE, 0:1],
                                                in1=hi2[:, 0:1], op=Alu.is_ge)
                        jf = scr.tile([B_CORE, 1], f32, tag="jf")
                        nc.vector.tensor_copy(jf[:], hi2[:, 1:2])
                        nc.vector.copy_predicated(out=jf[:], mask=win[:],
                                                  data=pair[:B_CORE, 1:2])
                        ju = scr.tile([B_CORE, 1], u32, tag="ju")
                        nc.vector.tensor_copy(ju[:], jf[:])

                    # gather W[j, :] and accumulate into y (DMA compute add)
                    if "gather" not in skip:
                        if "gstage" in skip:
                            wrow = scr.tile([B_CORE, N_IN], f32, tag="wrow")
                            nc.gpsimd.indirect_dma_start(
                                out=wrow[:], out_offset=None, in_=w_d[:],
                                in_offset=bass.IndirectOffsetOnAxis(
                                    ap=ju[:, 0:1], axis=0))
                            nc.vector.tensor_tensor(out=y[:], in0=y[:],
                                                    in1=wrow[:], op=Alu.add)
                        else:
                            nc.gpsimd.indirect_dma_start(
                                out=y[:], out_offset=None, in_=w_d[:],
                                in_offset=bass.IndirectOffsetOnAxis(
                                    ap=ju[:, 0:1], axis=0),
                                compute_op=Alu.add)

                    # flat pick index for the final encoded scatter
                    nc.vector.tensor_tensor(out=fpicks[:, t:t + 1], in0=ju[:],
                                            in1=rowbase[:], op=Alu.add)

                    # top-6 threshold of y
                    m8y = scr.tile([B_CORE, 8], f32, tag="m8y")
                    nc.vector.max(out=m8y[:], in_=y[:])
                    thr = m8y[:, K_IN - 1:K_IN]

                    if not last:
                        # xr mask (bf16); transpose via DMA xbar for next MM
                        xrb = scr.tile([B_CORE, N_IN], b16, tag="xrb")
                        nc.vector.tensor_tensor(
                            out=xrb[:], in0=y[:],
                            in1=thr.to_broadcast([B_CORE, N_IN]), op=Alu.is_ge)
                        if "transpose" not in skip:
                            for b in range(4):
                                pst = ptr.tile([128, 128], b16, tag="tr")
                                nc.tensor.transpose(
                                    pst[:, :B_CORE],
                                    xrb[:, 128 * b:128 * b + 128],
                                    ident_b[:B_CORE, :B_CORE])
                                nc.scalar.copy(out=xrT[:, b, :],
                                               in_=pst[:, :B_CORE])

                        # replicate j to both halves, mask r0pen at the pick
                        if "mask" not in skip:
                            jrep = scr.tile([128, 1], f32, tag="jrep")
                            nc.vector.tensor_copy(jrep[:B_CORE, :], jf[:])
                            nc.vector.tensor_copy(jrep[B_CORE:, :], jf[:])
                            nc.vector.scalar_tensor_tensor(
                                out=r0pen[:], in0=iota_f[:], scalar=jrep[:],
                                in1=r0pen[:], op0=Alu.not_equal, op1=Alu.mult)
                    else:
                        xrf = scr.tile([B_CORE, N_IN], f32, tag="xrf")
                        nc.vector.tensor_tensor(
                            out=xrf[:], in0=y[:],
                            in1=thr.to_broadcast([B_CORE, N_IN]), op=Alu.is_ge)
                        nc.sync.dma_start(out=xr_d[:], in_=xrf[:])

            full_loop_cm.__exit__(None, None, None)

            # ---------------- emit encoded ----------------
            for t in range(K_ACTIVE):
                nc.gpsimd.indirect_dma_start(
                    out=enc_d[:],
                    out_offset=bass.IndirectOffsetOnAxis(
                        ap=fpicks[:, t:t + 1], axis=0),
                    in_=ones[:], in_offset=None)

    nc.compile()
    return nc


def _get_nc(reps=1, skip=(), loop_scope="pursuit"):
    global _CACHED_NC
    if reps != 1 or skip:
        return _build(reps, skip, loop_scope)
    if _CACHED_NC is None:
        _CACHED_NC = _build()
    return _CACHED_NC


def kernel(x: np.ndarray, weight: np.ndarray):
    from concourse.bass_utils import run_bass_kernel_spmd

    x = np.ascontiguousarray(x, dtype=np.float32)
    weight = np.ascontiguousarray(weight, dtype=np.float32)
    nc = _get_nc()
    in_maps = [
        {"x": x[c * B_CORE:(c + 1) * B_CORE], "w": weight}
        for c in range(N_CORES)
    ]
    r = run_bass_kernel_spmd(nc, in_maps, list(range(N_CORES)))
    enc = np.concatenate(
        [r.results[c]["enc"].reshape(B_CORE, N_OUT) for c in range(N_CORES)],
        axis=0)
    xr = np.concatenate([r.results[c]["xr"] for c in range(N_CORES)], axis=0)
    return enc, xr
